# revision 32
# baseline (speedup 1.0000x reference)
"""Trainium2 Bass kernel for nn_AdaBoostClassifier (8-core data-parallel).

Reference computation:
    logits = x @ W.T + b                      # [N, E]
    preds  = round(sigmoid(logits))           # {0,1} == 1[logit > 0]
    acc    = sum_e trunc(alpha_e) * preds_e   # integer-valued
    out    = sign(acc)

Math used here: with t = trunc(alphas): acc = dot(t, preds) where
preds_e = 1[logit_e > 0] (is_gt matches round-half-even at the boundary).
Columns with t_e == 0 contribute nothing, so only those estimators are
computed (selected on host at runtime — valid for any input values).

Device pipeline per 512-sample block (default, DR=1):
  pass 1: xh=fp16(x) @ Wh=fp16(W) -> PSUM ps1 (fp32).
  pass C: one fp8e4 DoubleRow matmul (2 MACs/cell/cycle) computing
    xh8 @ (Wl*2^12) + ((x-xh)*2^12) @ Wh8  ->  PSUM pC = 2^12*(x@W - xh@Wh)
    with xh8 = fp8(xh) cast on-chip (split DVE/ACT); weights are paired
    per contraction row, loaded k-outer so the 256-col LDWEIGHTS is
    amortized over the sub-blocks.
  preds = 1[(ps1 + b) > -2^-12*pC] on DVE (ACT pre-scales pC).
  acc = t . preds via a PE matvec (t broadcast to 32 cols so two
  sub-blocks' accs land on adjacent PSUM partitions 31|32), one batched
  ACT Sign, DMA out. Total precision ~2^-16: 4 output flips in 131072
  vs the fp32 reference.

Fallback (n_etiles > 1): 3-pass mixed-precision matmul (xh@Wh fp16,
  xh@Wl bf16, xl_fp8@Ws with Ws=fp16(Wh*2^-12), all accumulating into
  one PSUM group) -> DVE is_gt -> matvec -> Sign.
"""

import math
import os

import numpy as np
import ml_dtypes

import concourse.bass as bass  # noqa: F401  (registers bass types)
import concourse.tile as tile
from concourse import bacc, mybir
from concourse.bass_utils import run_bass_kernel_spmd

BF16 = ml_dtypes.bfloat16
F8E4 = ml_dtypes.float8_e4m3
XL_FP8 = os.environ.get("KERNEL_XL_FP8", "1") == "1"
S_ON_DVE = os.environ.get("KERNEL_S_DVE", "1") == "1"
XBUFS = int(os.environ.get("KERNEL_XBUFS", "6"))
SPOOL = int(os.environ.get("KERNEL_SPOOL", "4"))
PSLOG = int(os.environ.get("KERNEL_PSLOG", "3"))
PEND = int(os.environ.get("KERNEL_PEND", "1"))
WARM = int(os.environ.get("KERNEL_WARM", "14"))
PSACC = int(os.environ.get("KERNEL_PSACC", "2"))
DR = os.environ.get("KERNEL_DR", "1") == "1"  # fp8 DoubleRow correction pass
XL_SCALE = 2.0 ** 12 if XL_FP8 else 1.0
XL_INV_SCALE = 1.0 / XL_SCALE
XL_DT = mybir.dt.float8e4 if XL_FP8 else mybir.dt.bfloat16
XL_NP = F8E4 if XL_FP8 else BF16

N_CORES = 8
N_FULL = 131072
F_DIM = 512
NS = N_FULL // N_CORES          # samples per core
BLK = 512                       # samples per psum block (one PSUM bank)
SUPER = int(os.environ.get("KERNEL_SUPER", "1024"))  # samples per DMA super-block
N_SUPER = NS // SUPER
SUBS = SUPER // BLK
KC = F_DIM // 128               # contraction chunks

_program_cache: dict[int, object] = {}


def _build(n_etiles: int):
    """Build the 8-core SPMD program for n_etiles 128-wide estimator tiles."""
    use_dr = DR and n_etiles == 1 and SUBS == 2
    nc = bacc.Bacc("TRN2", target_bir_lowering=False, debug=False)

    d_xh = nc.dram_tensor("xh", [F_DIM, NS], mybir.dt.float16, kind="ExternalInput")
    d_xl = nc.dram_tensor("xl", [F_DIM, NS], XL_DT, kind="ExternalInput")
    d_wh = nc.dram_tensor(
        "wh", [n_etiles, F_DIM, 128], mybir.dt.float16, kind="ExternalInput"
    )
    d_wl = nc.dram_tensor(
        "wl", [n_etiles, F_DIM, 128], mybir.dt.bfloat16, kind="ExternalInput"
    )
    d_ws = nc.dram_tensor(
        "ws", [n_etiles, F_DIM, 128], mybir.dt.float16, kind="ExternalInput"
    )
    d_w8 = nc.dram_tensor(
        "w8", [n_etiles, F_DIM, 2, 128], mybir.dt.float8e4, kind="ExternalInput"
    )
    d_bv = nc.dram_tensor("bv", [n_etiles, 128, 1], mybir.dt.float32, kind="ExternalInput")
    d_tt = nc.dram_tensor("tt", [1, 1], mybir.dt.float32, kind="ExternalInput")
    d_tv = nc.dram_tensor("tv", [n_etiles, 128, 1], mybir.dt.bfloat16, kind="ExternalInput")
    d_out = nc.dram_tensor("out", [NS], mybir.dt.float32, kind="ExternalOutput")

    # f-major views of x planes: (k p) n -> p k n, so chunk k = features
    # [128k, 128k+128) on partitions.
    xh_v = d_xh.ap().rearrange("(k p) n -> p k n", p=128)
    xl_v = d_xl.ap().rearrange("(k p) n -> p k n", p=128)
    out_v = d_out.ap().rearrange("(s n) -> s n", n=SUPER)
    out_v3 = d_out.ap().rearrange("(b s n) -> b s n", s=SUBS, n=BLK)

    with tile.TileContext(nc) as tc:
        with (
            tc.tile_pool(name="singles", bufs=1) as singles,
            tc.tile_pool(name="xbuf", bufs=XBUFS) as xbuf,
            tc.tile_pool(name="sbuf", bufs=SPOOL) as spool,
            tc.tile_pool(name="obuf", bufs=3) as obuf,
            tc.tile_pool(name="pslog", bufs=PSLOG, space="PSUM") as pslog,
            tc.tile_pool(name="psacc", bufs=PSACC if use_dr else 3, space="PSUM") as psacc,
            tc.tile_pool(name="psc", bufs=2 if use_dr else 1, space="PSUM") as psc,
            tc.tile_pool(name="pswarm", bufs=1, space="PSUM") as pswarm,
        ):
            # --- weights / per-estimator constants: few batched DMAs on the
            # ACT HWDGE ring so the SP ring streams x from cycle 0 ---
            wh_t = singles.tile([128, n_etiles, KC, 128], mybir.dt.float16, tag="wh")
            nc.scalar.dma_start(
                out=wh_t, in_=d_wh.ap().rearrange("j (k p) e -> p j k e", p=128)
            )
            wl_t = singles.tile([128, n_etiles, KC, 128], mybir.dt.bfloat16, tag="wl")
            nc.scalar.dma_start(
                out=wl_t, in_=d_wl.ap().rearrange("j (k p) e -> p j k e", p=128)
            )
            ws_t = singles.tile([128, n_etiles, KC, 128], mybir.dt.float16, tag="ws")
            nc.scalar.dma_start(
                out=ws_t, in_=d_ws.ap().rearrange("j (k p) e -> p j k e", p=128)
            )
            bv_t = singles.tile([128, n_etiles], mybir.dt.float32, tag="bv")
            nc.scalar.dma_start(
                out=bv_t, in_=d_bv.ap().rearrange("j p one -> p (j one)")
            )
            tv_t = singles.tile([128, n_etiles], mybir.dt.bfloat16, tag="tv")
            nc.scalar.dma_start(
                out=tv_t, in_=d_tv.ap().rearrange("j p one -> p (j one)")
            )
            tt_t = singles.tile([1, 1], mybir.dt.float32, tag="tt")
            nc.scalar.dma_start(out=tt_t, in_=d_tt.ap())
            if use_dr:
                w8_t = singles.tile(
                    [128, n_etiles, KC, 2, 128], mybir.dt.float8e4, tag="w8"
                )
                nc.scalar.dma_start(
                    out=w8_t,
                    in_=d_w8.ap().rearrange("j (k p) two e -> p j k two e", p=128),
                )
                # t broadcast to 32 columns: the per-sub-block matvec writes
                # 32 replicated partitions so rows 31|32 of the shared acc
                # bank are adjacent -> one dense-partition Sign for SUBS=2
                tvb_t = singles.tile([128, 32], mybir.dt.bfloat16, tag="tvb")
                nc.vector.tensor_copy(
                    out=tvb_t, in_=tv_t[:, 0:1].to_broadcast((128, 32))
                )

            # PE warmup: dummy matmuls on a memset tile while the first x
            # super-block streams in — ramps the HAM clock gate to 2.4 GHz
            # so the real matmuls start warm. No data dependencies.
            if WARM:
                wsrc = singles.tile([128, BLK], mybir.dt.bfloat16, tag="wsrc")
                nc.vector.memset(wsrc, 0.0)
                warmp = pswarm.tile([128, BLK], mybir.dt.float32, tag="warmp")
                for _ in range(WARM):
                    nc.tensor.matmul(
                        warmp, wsrc[:, 0:128], wsrc, start=True, stop=True
                    )

            # --- main loop ---
            # Stage-2 (t-matvec + output Sign + out-DMA) for sub-block i is
            # emitted while sub-block i+1's stage-1 matmuls run, so the PE
            # never idles waiting for ACT's Sign output.
            pending = []  # (s_tiles per j, out_sb, ns, sb, s) awaiting stage 2
            out_tiles = {}
            acc_tiles = {}

            def flush_stage2():
                s_tiles, o_sb, ns_, _sb, _s = pending.pop(0)
                if use_dr:
                    # matvecs of one super-block accumulate into 32-aligned
                    # partitions of a shared PSUM tile; one batched Sign.
                    if _s == 0:
                        acc_tiles[_sb] = psacc.tile(
                            [128, BLK], mybir.dt.float32, tag="acc",
                            name=f"acc{_sb}")
                    acc = acc_tiles[_sb]
                    row = 32 * _s
                    nc.tensor.matmul(
                        acc[row:row + 32, :], tvb_t, s_tiles[0],
                        start=True, stop=True,
                    )
                    if _s == SUBS - 1:
                        nc.scalar.activation(
                            out=o_sb, in_=acc[0:32 * SUBS, :],
                            func=mybir.ActivationFunctionType.Sign,
                        )
                        nc.scalar.dma_start(
                            out=out_v3[_sb],
                            in_=o_sb[31:31 + SUBS, :],
                        )
                    return
                acc = psacc.tile([1, BLK], mybir.dt.float32, tag="acc",
                                 name=f"acc{_sb}_{ns_.start}")
                for j in range(n_etiles):
                    nc.tensor.matmul(
                        acc, tv_t[:, j:j + 1], s_tiles[j],
                        start=(j == 0), stop=(j == n_etiles - 1),
                    )
                nc.scalar.activation(
                    out=o_sb[0:1, ns_], in_=acc,
                    func=mybir.ActivationFunctionType.Sign,
                    bias=tt_t,
                )
                if _sb == N_SUPER - 1:
                    nc.scalar.dma_start(
                        out=out_v[_sb, ns_], in_=o_sb[0:1, ns_]
                    )

            for sb in range(N_SUPER):
                n0 = sb * SUPER
                xh_sb = xbuf.tile([128, KC, SUPER], mybir.dt.float16, tag="xh",
                                  name=f"xh{sb}")
                if use_dr:
                    x8_sb = xbuf.tile([128, KC, 2, SUPER], mybir.dt.float8e4,
                                      tag="x8", name=f"x8{sb}")
                    xl_dst = x8_sb[:, :, 1, :]
                else:
                    xl_sb = xbuf.tile([128, KC, SUPER], XL_DT, tag="xl",
                                      name=f"xl{sb}")
                    xl_dst = xl_sb
                if sb == 0:
                    for s in range(0, SUBS):
                        lo, hi = s * BLK, (s + 1) * BLK
                        nc.sync.dma_start(
                            out=xh_sb[:, :, lo:hi],
                            in_=xh_v[:, :, n0 + lo:n0 + hi])
                        nc.sync.dma_start(
                            out=xl_dst[:, :, lo:hi],
                            in_=xl_v[:, :, n0 + lo:n0 + hi])
                else:
                    nc.sync.dma_start(out=xh_sb, in_=xh_v[:, :, n0:n0 + SUPER])
                    nc.sync.dma_start(out=xl_dst, in_=xl_v[:, :, n0:n0 + SUPER])
                if use_dr:
                    # fill pair plane 0 with fp8(xh); split across DVE and
                    # ACT (GpSimd's Q7 fp8 conversion is far too slow)
                    half = KC // 2
                    nc.vector.tensor_copy(
                        out=x8_sb[:, 0:half, 0, :], in_=xh_sb[:, 0:half, :])
                    nc.scalar.activation(
                        out=x8_sb[:, half:KC, 0, :], in_=xh_sb[:, half:KC, :],
                        func=mybir.ActivationFunctionType.Copy)

                if use_dr:
                    out_sb = obuf.tile([32 * SUBS, BLK], mybir.dt.float32,
                                       tag="osb", name=f"osb{sb}")
                else:
                    out_sb = obuf.tile([1, SUPER], mybir.dt.float32, tag="osb",
                                       name=f"osb{sb}")
                out_tiles[sb] = out_sb

                if use_dr:
                    ps1 = {}
                    pCs = {}
                    # pass C FIRST (k-outer: each DoubleRow weight load
                    # serves all SUBS matmuls, hiding the 256-col
                    # LDWEIGHTS); pass 1 then runs while pC's epilogue
                    # chain (ACT scale-copy -> DVE is_gt) drains, so the
                    # matvec never stalls the PE.
                    for s in range(SUBS):
                        pCs[s] = psc.tile([128, BLK], mybir.dt.float32,
                                          tag="pc", name=f"pc{sb}_{s}")
                    for k in range(KC):
                        for s in range(SUBS):
                            ns = slice(s * BLK, (s + 1) * BLK)
                            nc.tensor.matmul(
                                pCs[s], w8_t[:, 0, k, :, :],
                                x8_sb[:, k, :, ns],
                                start=(k == 0), stop=(k == KC - 1),
                                perf_mode=mybir.MatmulPerfMode.DoubleRow,
                            )
                    for s in range(SUBS):
                        ns = slice(s * BLK, (s + 1) * BLK)
                        lg = pslog.tile([128, BLK], mybir.dt.float32, tag="lg",
                                        name=f"lg{sb}_{s}")
                        ps1[s] = lg
                        for k in range(KC):
                            nc.tensor.matmul(
                                lg, wh_t[:, 0, k, :], xh_sb[:, k, ns],
                                start=(k == 0), stop=(k == KC - 1),
                            )
                    for s in range(SUBS):
                        ns = slice(s * BLK, (s + 1) * BLK)
                        s_t = spool.tile([128, BLK], mybir.dt.bfloat16,
                                         tag="sg", name=f"sg{sb}_{s}")
                        # logit = ps1 + b + 2^-12*pC; pred = 1[logit > 0]
                        #       = 1[(ps1 + b) > -2^-12*pC]   (bv holds +b)
                        t0 = spool.tile([128, BLK], mybir.dt.float32,
                                        tag="t0", name=f"t0{sb}_{s}")
                        nc.vector.tensor_scalar_mul(t0, pCs[s], -XL_INV_SCALE)
                        nc.vector.scalar_tensor_tensor(
                            out=s_t, in0=ps1[s], scalar=bv_t[:, 0:1],
                            in1=t0, op0=mybir.AluOpType.add,
                            op1=mybir.AluOpType.is_gt,
                        )
                        pending.append(([s_t], out_sb, ns, sb, s))
                        if len(pending) > PEND:
                            flush_stage2()
                    if sb > 0 and not use_dr:
                        nc.scalar.dma_start(
                            out=out_v[sb - 1:sb, :], in_=out_tiles[sb - 1])
                    continue

                for s in range(SUBS):
                    ns = slice(s * BLK, (s + 1) * BLK)
                    s_tiles = []
                    for j in range(n_etiles):
                        logits = pslog.tile([128, BLK], mybir.dt.float32, tag="lg",
                                            name=f"lg{sb}_{s}_{j}")
                        s_t = spool.tile([128, BLK], mybir.dt.bfloat16, tag="sg",
                                         name=f"sg{sb}_{s}_{j}")
                        if True:
                            # xh-dependent passes first so the PE can start
                            # as soon as xh lands, while xl still streams in.
                            passes = [(wh_t, xh_sb), (wl_t, xh_sb), (ws_t, xl_sb)]
                            mm = 0
                            for w_t, x_sb in passes:
                                for k in range(KC):
                                    nc.tensor.matmul(
                                        logits, w_t[:, j, k, :], x_sb[:, k, ns],
                                        start=(mm == 0), stop=(mm == 3 * KC - 1),
                                    )
                                    mm += 1
                            # S01 = 1[L > -b] = pred in {0,1}; acc = t . S01
                            if S_ON_DVE:
                                # S01 = 1[L > -b]; bv holds -b
                                nc.vector.tensor_scalar(
                                    s_t, logits, bv_t[:, j:j + 1], None,
                                    mybir.AluOpType.is_gt,
                                )
                            else:
                                # S = Sign(L + b); bv holds +b and the final
                                # Sign gets bias T = sum(t)
                                nc.scalar.activation(
                                    out=s_t, in_=logits,
                                    func=mybir.ActivationFunctionType.Sign,
                                    bias=bv_t[:, j:j + 1],
                                )
                        s_tiles.append(s_t)
                    pending.append((s_tiles, out_sb, ns, sb, s))
                    if len(pending) > PEND:
                        flush_stage2()
                # previous super-block's outputs are all signed by now
                if sb > 0:
                    nc.scalar.dma_start(
                        out=out_v[sb - 1:sb, :], in_=out_tiles[sb - 1]
                    )
            while pending:
                flush_stage2()

    nc.compile()
    return nc


def _prep_inputs(x, W, b, alphas):
    """Host-side prep: estimator selection, transposes, hi/lo splits."""
    t_full = np.trunc(alphas.astype(np.float32)).astype(np.float32)
    T = float(t_full.sum())
    nz = np.flatnonzero(t_full)
    n_etiles = max(1, math.ceil(len(nz) / 128))
    e_pad = n_etiles * 128

    W_sel = np.zeros((e_pad, F_DIM), np.float32)
    b_sel = np.zeros((e_pad,), np.float32)
    t_sel = np.zeros((e_pad,), np.float32)
    if len(nz):
        W_sel[: len(nz)] = W[nz]
        b_sel[: len(nz)] = b[nz]
        t_sel[: len(nz)] = t_full[nz]

    # [n_etiles, F, 128] stationary layout (partition = feature)
    w_fe = W_sel.T.reshape(F_DIM, n_etiles, 128).transpose(1, 0, 2)
    wh = w_fe.astype(np.float16)
    wl = (w_fe - wh.astype(np.float32)).astype(BF16)
    # scaled copy of wh for the fp8 xl pass: products (xl*2^12)*(wh*2^-12)
    # accumulate unscaled into the same PSUM group. Subnormal fp16 is fine
    # (PE handles it; pass-3's W only needs ~8 bits since xl ~ 2^-12 |x|).
    ws = (wh.astype(np.float32) * XL_INV_SCALE).astype(np.float16)

    xT = np.ascontiguousarray(x.T.astype(np.float32))  # [F, N]
    xh = xT.astype(np.float16)
    xl = ((xT - xh.astype(np.float32)) * XL_SCALE).astype(XL_NP)

    wl32 = w_fe - wh.astype(np.float32)
    w8 = np.empty((n_etiles, F_DIM, 2, 128), F8E4)
    w8[..., 0, :] = (wl32 * XL_SCALE).astype(F8E4)
    w8[..., 1, :] = wh.astype(np.float32).astype(F8E4)

    if (DR and n_etiles == 1) or not S_ON_DVE:
        bv = np.ascontiguousarray(b_sel.reshape(n_etiles, 128, 1))
    else:
        bv = np.ascontiguousarray((-b_sel).reshape(n_etiles, 128, 1))
    if S_ON_DVE:
        tt = np.array([[0.0]], np.float32)
    else:
        tt = np.array([[float(t_sel.sum())]], np.float32)
    tv = np.ascontiguousarray(t_sel.reshape(n_etiles, 128, 1)).astype(BF16)

    in_maps = []
    for c in range(N_CORES):
        sl = slice(c * NS, (c + 1) * NS)
        in_maps.append({
            "xh": np.ascontiguousarray(xh[:, sl]),
            "xl": np.ascontiguousarray(xl[:, sl]),
            "wh": wh, "wl": wl, "ws": ws, "w8": w8, "bv": bv, "tv": tv,
            "tt": tt,
        })
    return n_etiles, in_maps


def kernel(x, W, b, alphas, _trace=False, _trace_kwargs=None):
    n_etiles, in_maps = _prep_inputs(
        np.asarray(x), np.asarray(W), np.asarray(b), np.asarray(alphas)
    )
    cache_key = (n_etiles, XL_FP8, S_ON_DVE, SUPER, XBUFS, SPOOL, PSLOG, PEND, WARM, DR, PSACC)
    nc = _program_cache.get(cache_key)
    if nc is None:
        nc = _build(n_etiles)
        _program_cache[cache_key] = nc

    kwargs = {}
    if _trace:
        kwargs["trace"] = True
        kwargs.update(_trace_kwargs or {})
    res = run_bass_kernel_spmd(nc, in_maps, core_ids=list(range(N_CORES)), **kwargs)
    out = np.concatenate([res.results[c]["out"] for c in range(N_CORES)])
    if _trace:
        kernel.last_results = res
    return out.astype(np.float32)
